# revision 39
# baseline (speedup 1.0000x reference)
"""Trainium2 Bass kernel for nn_BasicLSTM: fc0 -> 10x LSTM(768) -> fc1.

Strategy: data-parallel over the 512 windows across 8 cores (64 windows each).
All matmul operands in bf16 (f32 PSUM accumulation, f32 cell state).

I/O: the ~107MB of weights are embedded in the NEFF as a Const tensor
(inline_tensor) and DMA'd to HBM once at model load; the only runtime input
is the transposed window positions (~1MB/core). Per-execute dispatch
overhead in this runtime scales with the number/size of argument buffers
(~0.85ms per large buffer, measured), so keeping weights out of the
argument list removes nearly all of it. The program is cached keyed on an
md5 of the packed weights.

Device schedule (per core, fused):
  The LSTM recurrence serializes on PE (h_{t-1} @ w_hhT per step, stationary
  = 64-wide h^T block, moving = 512-wide w_hh chunks) with ACT/DVE gate and
  state tails in between; the input GEMM for the NEXT layer (gx = X^T.T @
  w_ihT + bias, batched over all timesteps) is emitted in units interleaved
  between a step's matmuls and its h-store so the in-order PE queue runs gx
  during the tails:
    wave A (m-tiles 0,1 = steps 0..3) after rec steps 4..9 of layer l,
    wave B (m-tiles 2,3,4 = steps 4..9) at the layer boundary and after
    rec steps 0..3 of layer l+1,
    fc1 k-tile GEMMs play the same role during the last layer (one step
    delayed so they never wait on the just-written X^T column).
  gx lives as a single (128, mt, g4) tile: partition p holds batch row
  p%64 of step 2m + p//64. Even steps read their rows at partition base 0
  (PSUM + gx via a DVE add); odd steps initialize the PSUM accumulation
  group with a constant (128,64) selector matmul whose transpose picks gx
  rows 64-127 — engines cannot partition-shift, but a stationary selector
  can, and it rides the PE accumulation for free. Gate nonlinearities are
  applied per chunk (ACT reads PSUM directly on the odd path).
  w_ih and w_hh are each held as full-layer resident SBUF tiles (36KB/
  partition, single-buffered; handoff windows: w_ih^{l+1} loads at layer
  l's step-3 slot right after the last wave-B unit releases w_ih^l, w_hh
  at the boundary, each split into 12 DMAs in the recurrence's chunk order
  for queue parallelism).
  Rec gate chunks are processed in order (g,g|o,i,i|f,f,o) and ACT ops
  emitted in operand-arrival order so the o-gate (shortest remaining
  chain) lands last; PSUM->SBUF gx copies run on ACT; elementwise on DVE.
fc0 is folded into layer 0's weights on the host (its output
feeds only gx0). Sim (TimelineSim) makespan 1.32ms/core, PE ~94% busy.
"""
import numpy as np
import ml_dtypes

H = 768
G = 4 * H          # 3072
W = 10             # time steps (window size)
L = 10             # layers
B_FULL = 512
NCORES = 8
BL = B_FULL // NCORES  # 64 windows per core

# ---- packed-input layout (bf16 element offsets) ----
_SIZES = [
    ("xposT", H * W * BL),        # (H, W*BL)
    ("fc0wT", H * H),             # (H, H)
    ("fc0b", H),                  # (1, H)
    ("wihT", L * H * G),          # (L, H, G)
    ("whhT", L * H * G),          # (L, H, G)
    ("biasT", L * G),             # (L, 1, G)
    ("fc1wT", W * H * H),         # (W*H, H)
    ("fc1bT", H),                 # (1, H)
]
OFF = {}
_c = 0
for _n, _s in _SIZES:
    OFF[_n] = _c
    _c += _s
TOTAL_ELEMS = _c
W_BASE = OFF["fc0wT"]          # weight region starts after xposT
W_ELEMS = TOTAL_ELEMS - W_BASE

_CACHE = {}


def build_program(weights_packed, h=H, w=W, nl=L, bl=BL):
    import concourse.mybir as mybir
    import concourse.tile as tile
    from concourse import bacc
    from concourse.masks import make_identity

    F32 = mybir.dt.float32
    BF16 = mybir.dt.bfloat16
    AF = mybir.ActivationFunctionType
    OP = mybir.AluOpType

    g4 = 4 * h
    kt = h // 128           # k-tiles over h
    nch = g4 // 512         # 512-wide chunks over the gate dim
    mt = (w * bl) // 128    # m-tiles over the (t, b) axis
    fn1 = h // 2            # fc1 output chunk (two psum chunks)
    REC_J_ORDER = (3, 4, 0, 1, 2, 5)   # g chunks first, o last
    assert h % 128 == 0 and g4 % 512 == 0 and (w * bl) % 128 == 0 and bl == 64

    nc = bacc.Bacc("TRN2", target_bir_lowering=False, debug=False)

    # weights are embedded in the NEFF as a Const tensor (DMA'd to HBM once
    # at model load); only xposT is a runtime input, so per-execute argument
    # staging is ~1MB/core instead of ~108MB/core.
    assert weights_packed is not None and weights_packed.size == W_ELEMS
    pk = nc.inline_tensor(
        np.ascontiguousarray(weights_packed, ml_dtypes.bfloat16), name="wconst")
    xpos_d = nc.dram_tensor("xpos", [h * w * bl], BF16, kind="ExternalInput")
    out_d = nc.dram_tensor("out", [bl, h], F32, kind="ExternalOutput")

    def pv(name, n, pattern=None, **axes):
        base = OFF[name] - W_BASE
        ap = pk[base:base + n]
        return ap.rearrange(pattern, **axes) if pattern else ap

    xposT_v = xpos_d[:].rearrange("(k p c) -> p k c", p=128, c=w * bl)
    fc1bT_v = pv("fc1bT", h, "(a ho) -> a ho", a=1)

    def wihT_v(l, k, js):
        base = OFF["wihT"] - W_BASE + l * h * g4
        ap = pk[base:base + h * g4].rearrange("(k p g) -> p k g", p=128, g=g4)
        return ap[:, k, js]

    def whhT_v(l):
        base = OFF["whhT"] - W_BASE + l * h * g4
        return pk[base:base + h * g4].rearrange("(k p g) -> p k g", p=128, g=g4)

    def biasT_v(l):
        base = OFF["biasT"] - W_BASE + l * g4
        return pk[base:base + g4].rearrange("(a g) -> a g", a=1)

    def fc1wT_v(t):
        base = OFF["fc1wT"] - W_BASE + t * h * h
        return pk[base:base + h * h].rearrange("(s p c) -> p s c", p=128, c=h)

    with tile.TileContext(nc) as tc, \
         tc.tile_pool(name="persist", bufs=1) as pp, \
         tc.tile_pool(name="whhp", bufs=1) as whhp, \
         tc.tile_pool(name="wihp", bufs=1) as wihp, \
         tc.tile_pool(name="gxpool", bufs=2) as gxpool, \
         tc.tile_pool(name="biasp", bufs=2) as biasp, \
         tc.tile_pool(name="wstream", bufs=6) as wsp, \
         tc.tile_pool(name="gpool", bufs=1) as gp, \
         tc.tile_pool(name="gatep", bufs=1) as gatep, \
         tc.tile_pool(name="tmp", bufs=1) as tp, \
         tc.tile_pool(name="cpool", bufs=2) as cp, \
         tc.tile_pool(name="hpool", bufs=1) as hp, \
         tc.tile_pool(name="psR", bufs=3, space="PSUM") as psR, \
         tc.tile_pool(name="psG", bufs=2, space="PSUM") as psG, \
         tc.tile_pool(name="psT", bufs=1, space="PSUM") as psT, \
         tc.tile_pool(name="psF", bufs=2, space="PSUM") as psF:

        # ---- persistent tiles ----
        XT = pp.tile([128, kt, w * bl], BF16)      # h^T / layer-input storage
        onesb = pp.tile([1, 512], BF16)
        nc.vector.memset(onesb[:], 1.0)
        idb = pp.tile([64, 64], BF16)
        make_identity(nc, idb[:])
        # (128, 64) selectors: stationary operands whose transpose picks the
        # even (rows 0-63) / odd (rows 64-127) half of a 128-partition gx
        # m-tile while initializing the PSUM accumulation group -- replaces
        # both a DVE add and the odd-step partition-shift DMA.
        sel_eo = [pp.tile([128, 64], BF16, name=f"sel{half}") for half in range(2)]
        for half in range(2):
            nc.vector.memset(sel_eo[half][:], 0.0)
            make_identity(nc, sel_eo[half][half * 64:(half + 1) * 64, :],
                          nomemset=True)
        fc1b_sb = pp.tile([1, h], BF16)
        nc.sync.dma_start(fc1b_sb[:], fc1bT_v)

        def load_whh(l):
            t = whhp.tile([128, kt, g4], BF16, tag="whh", name=f"whh{l}")
            v = whhT_v(l)
            hk = kt // 2
            for j in REC_J_ORDER:   # first-needed gate chunks land first
                js = slice(j * 512, (j + 1) * 512)
                for kh in range(2):
                    ks = slice(kh * hk, (kh + 1) * hk)
                    nc.sync.dma_start(t[:, ks, js], v[:, ks, js])
            return t

        def load_wih(l, fine_first=False):
            t = wihp.tile([128, kt, g4], BF16, tag="wih", name=f"wih{l}")
            base = OFF["wihT"] - W_BASE + l * h * g4
            v = pk[base:base + h * g4].rearrange("(k p g) -> p k g", p=128, g=g4)
            hk = kt // 2
            for j in range(nch):
                js = slice(j * 512, (j + 1) * 512)
                if fine_first and j == 0:
                    # per-k DMAs so the first consumer matmul starts sooner
                    for k in range(kt):
                        nc.sync.dma_start(t[:, k, js], v[:, k, js])
                    continue
                for kh in range(2):
                    ks = slice(kh * hk, (kh + 1) * hk)
                    nc.sync.dma_start(t[:, ks, js], v[:, ks, js])
            return t

        def load_bias(l):
            t = biasp.tile([1, g4], BF16, tag="bias", name=f"bias{l}")
            nc.sync.dma_start(t[:], biasT_v(l))
            return t

        def emit_gx_unit(wih_res, ms, j, gx_dst, bias_sb, uname, src=None):
            """gx_dst[:, m, js] = src[:, :, m-cols].T @ wih_res[:, js] + bias."""
            if src is None:
                src = XT
            js = slice(j * 512, (j + 1) * 512)
            for m in ms:
                ps = psG.tile([128, 512], F32, tag="gxw",
                              name=f"gxps_{uname}_{j}_{m}")
                for k in range(kt):
                    nc.tensor.matmul(
                        ps[:], src[:, k, m * 128:(m + 1) * 128], wih_res[:, k, js],
                        start=(k == 0), stop=False)
                nc.tensor.matmul(
                    ps[:], onesb[:, 0:128], bias_sb[:, js], start=False, stop=True)
                nc.scalar.activation(gx_dst[:, m, js], ps[:], AF.Copy)

        def rec_step_compute(l, t, m, half, gx_cur, whh_cur, c_cur):
            """One LSTM step's gates/state; returns (c_new, hh).

            t >= 1: per gate chunk j, PSUM is initialized with the step's gx
            rows via a selector matmul (stationary sel_eo[half], moving
            gx_cur[:, m, js]), h_{t-1} @ w_hh accumulates on top, and ACT
            applies the gate nonlinearity reading PSUM directly. Chunks are
            processed g-first / o-last (the o-gate feeds the shortest
            remaining chain); the c chain runs on DVE.
            """
            gates = gatep.tile([64, g4], BF16, tag="gates", name=f"gates_{l}_{t}")
            if t == 0:
                gx_t = gx_cur[0:64, m, :]
                nc.scalar.activation(gates[:, 0:2 * h], gx_t[:, 0:2 * h], AF.Sigmoid)
                nc.scalar.activation(gates[:, 2 * h:3 * h], gx_t[:, 2 * h:3 * h],
                                     AF.Tanh)
                nc.scalar.activation(gates[:, 3 * h:4 * h], gx_t[:, 3 * h:4 * h],
                                     AF.Sigmoid)
            else:
                # even steps need no partition shift: a DVE add is cheaper
                # for PE (the bottleneck); odd steps get the selector matmul
                # which shifts and adds in one PE op.
                g_sb = None
                if half == 0:
                    g_sb = gp.tile([64, g4], BF16, tag="g", name=f"g_{l}_{t}")
                for j in REC_J_ORDER:
                    js = slice(j * 512, (j + 1) * 512)
                    ps = psR.tile([128, 512], F32, tag="recps",
                                  name=f"recps_{l}_{t}_{j}")
                    if half == 1:
                        nc.tensor.matmul(
                            ps[0:64, :], sel_eo[1][:], gx_cur[:, m, js],
                            start=True, stop=False)
                    for k in range(kt):
                        nc.tensor.matmul(
                            ps[0:64, :],
                            XT[:, k, (t - 1) * 64:t * 64],
                            whh_cur[:, k, js],
                            start=(half == 0 and k == 0), stop=(k == kt - 1))
                    if half == 0:
                        nc.vector.tensor_tensor(
                            g_sb[:, js], ps[0:64, :], gx_cur[0:64, m, js], OP.add)
                    # gate nonlinearity per chunk, split at function bounds
                    c0, c1 = j * 512, (j + 1) * 512
                    for f0, f1, fn in ((0, h, AF.Sigmoid), (h, 2 * h, AF.Sigmoid),
                                       (2 * h, 3 * h, AF.Tanh),
                                       (3 * h, 4 * h, AF.Sigmoid)):
                        a0, a1 = max(c0, f0), min(c1, f1)
                        if a0 < a1:
                            src = (g_sb[:, a0:a1] if half == 0
                                   else ps[0:64, a0 - c0:a1 - c0])
                            nc.scalar.activation(gates[:, a0:a1], src, fn)
            c_new = cp.tile([64, h], F32, tag="c", name=f"c_{l}_{t}")
            if t == 0:
                nc.vector.tensor_tensor(
                    c_new[:], gates[:, 0:h], gates[:, 2 * h:3 * h], OP.mult)
            else:
                t1 = tp.tile([64, h], BF16, tag="t1")
                nc.vector.tensor_tensor(
                    t1[:], gates[:, 0:h], gates[:, 2 * h:3 * h], OP.mult)
                t2 = tp.tile([64, h], BF16, tag="t2")
                nc.vector.tensor_tensor(t2[:], gates[:, h:2 * h], c_cur[:], OP.mult)
                nc.vector.tensor_tensor(c_new[:], t1[:], t2[:], OP.add)
            tc_t = tp.tile([64, h], BF16, tag="tc")
            nc.scalar.activation(tc_t[:], c_new[:], AF.Tanh)
            hh = hp.tile([64, h], BF16, tag="hh", name=f"hh_{l}_{t}")
            nc.vector.tensor_tensor(hh[:], gates[:, 3 * h:4 * h], tc_t[:], OP.mult)
            return c_new, hh

        def rec_step_store(t, hh):
            """Transpose h back into XT (emitted after the tail-filling unit
            so PE's in-order queue runs the unit during the gate/state tail)."""
            trp = psT.tile([128, kt * 64], BF16, tag="trp")
            for s in range(kt):
                nc.tensor.transpose(
                    trp[:, s * 64:(s + 1) * 64], hh[:, s * 128:(s + 1) * 128], idb[:])
            nc.vector.tensor_copy(
                XT[:, :, t * 64:(t + 1) * 64],
                trp[:].rearrange("p (s x) -> p s x", s=kt))

        # ---- layer 0 prologue: gx0 straight from x (fc0 folded into
        # wih[0]/bias[0] on the host), whh0, bias0 ----
        bias_cur = load_bias(0)
        xpt = gxpool.tile([128, kt, w * bl], BF16, tag="gx", name="xpt")
        for k in range(kt):
            nc.sync.dma_start(xpt[:, k, :], xposT_v[:, k, :])
        wih_cur = load_wih(0, fine_first=True)
        whh_cur = load_whh(0)
        gx_cur = gxpool.tile([128, mt, g4], BF16, tag="gx", name="gx0")
        for j in range(nch):
            emit_gx_unit(wih_cur, range(mt), j, gx_cur, bias_cur, "l0", src=xpt)

        # ---- fc1 helpers (interleaved into layer nl-1's recurrence) ----
        psf = [psF.tile([128, 512], F32, tag="fc1acc", name=f"fc1ps_{n}")
               for n in range(2)]
        fc1w_tiles = {}

        def fc1_dma(t):
            wt = wsp.tile([128, kt, h], BF16, tag="fc1w", name=f"fc1w_{t}", bufs=2)
            v = fc1wT_v(t)
            nc.sync.dma_start(wt[:, 0:kt // 2, :], v[:, 0:kt // 2, :])
            nc.sync.dma_start(wt[:, kt // 2:kt, :], v[:, kt // 2:kt, :])
            fc1w_tiles[t] = wt

        def fc1_unit(t):
            wt = fc1w_tiles.pop(t)
            for n in range(2):
                ns = slice(n * fn1, (n + 1) * fn1)
                for s in range(kt):
                    nc.tensor.matmul(
                        psf[n][0:64, :fn1], XT[:, s, t * 64:(t + 1) * 64],
                        wt[:, s, ns], start=(t == 0 and s == 0), stop=False)

        # ---- layers (rec fused with next layer's gx waves / fc1) ----
        pendB = None  # (gx tile, bias tile, layer) with units j=2..5 pending
        for l in range(nl):
            last = l == nl - 1
            if not last:
                bias_nx = load_bias(l + 1)
                gx_next = gxpool.tile([128, mt, g4], BF16, tag="gx",
                                      name=f"gx{l + 1}")
            else:
                fc1_dma(0)
            c_cur = None
            for t in range(w):
                c_cur, hh = rec_step_compute(
                    l, t, t // 2, t % 2, gx_cur, whh_cur, c_cur)
                # tail-filling PE work, emitted before the h store so the
                # in-order PE queue runs it while ACT/DVE compute the gates
                if pendB is not None and t <= 3:
                    gxB, biasB, lB = pendB
                    emit_gx_unit(wih_cur, (2, 3, 4), 2 + t, gxB, biasB,
                                 f"B{lB}")
                    if t == 3:
                        pendB = None
                if t == 3 and not last:
                    wih_nx = load_wih(l + 1)
                if not last and t >= 4:
                    emit_gx_unit(wih_nx, (0, 1), t - 4, gx_next, bias_nx,
                                 f"A{l + 1}")
                if last:
                    if t < w - 1:
                        fc1_dma(t + 1)
                    if t >= 1:
                        fc1_unit(t - 1)  # one step delayed: XT col ready
                rec_step_store(t, hh)
            if not last:
                whh_next = load_whh(l + 1)
                emit_gx_unit(wih_nx, (2, 3, 4), 0, gx_next, bias_nx, f"B{l + 1}")
                emit_gx_unit(wih_nx, (2, 3, 4), 1, gx_next, bias_nx, f"B{l + 1}")
                pendB = (gx_next, bias_nx, l + 1)
                gx_cur, whh_cur, bias_cur = gx_next, whh_next, bias_nx
                wih_cur = wih_nx

        # ---- fc1 tail ----
        fc1_unit(w - 1)
        out_sb = pp.tile([64, h], F32)
        for n in range(2):
            ns = slice(n * fn1, (n + 1) * fn1)
            nc.tensor.matmul(
                psf[n][0:64, :fn1], onesb[:, 0:64], fc1b_sb[:, ns],
                start=False, stop=True)
            nc.vector.tensor_copy(out_sb[:, ns], psf[n][0:64, :fn1])
        nc.sync.dma_start(out_d[:], out_sb[:])

    nc.compile()
    return nc


def _pack_shared(inputs, h=H, w=W, nl=L):
    """Weights part of the packed buffer (identical across cores), bf16.

    fc0 is folded into layer 0 on the host: since fc0's output feeds only
    layer 0's input GEMM, gx0 = (x @ fc0w.T + fc0b) @ wih0.T + b0
    = x @ (fc0w.T @ wih0.T) + (fc0b @ wih0.T + b0). The packed layout keeps
    the (now unused) fc0 slots so offsets stay fixed.
    """
    f32 = np.float32
    bf16 = ml_dtypes.bfloat16
    fc0wT = np.ascontiguousarray(np.asarray(inputs["fc0_w"], f32).T)
    fc0b = np.asarray(inputs["fc0_b"], f32)
    wihT = np.ascontiguousarray(
        np.asarray(inputs["w_ih"], f32).transpose(0, 2, 1)).copy()
    bias = (np.asarray(inputs["b_ih"], f32) + np.asarray(inputs["b_hh"], f32)).copy()
    bias[0] = bias[0] + fc0b @ wihT[0]
    wihT[0] = fc0wT @ wihT[0]
    parts = [
        fc0wT.astype(bf16).ravel(),
        fc0b.astype(bf16).ravel(),
        wihT.astype(bf16).ravel(),
        np.ascontiguousarray(
            np.asarray(inputs["w_hh"], f32).transpose(0, 2, 1)).astype(bf16).ravel(),
        bias.astype(bf16).ravel(),
        np.ascontiguousarray(np.asarray(inputs["fc1_w"], f32).T).astype(bf16).ravel(),
        np.asarray(inputs["fc1_b"], f32).astype(bf16).ravel(),
    ]
    return np.concatenate(parts)


def prep_inputs(inputs):
    """Per-core runtime inputs: the transposed window positions only."""
    bf16 = ml_dtypes.bfloat16
    x = np.ascontiguousarray(
        np.asarray(inputs["x_position"], np.float32)).reshape(-1, W, H)
    in_maps = []
    for c in range(NCORES):
        xc = x[c * BL:(c + 1) * BL]
        xposT = np.ascontiguousarray(
            xc.transpose(2, 1, 0).reshape(H, W * BL)).astype(bf16).ravel()
        in_maps.append({"xpos": xposT})
    return in_maps


def get_nc(inputs):
    """Build (or reuse) the program with these weights baked in as Consts."""
    import hashlib
    wp = _pack_shared(inputs)
    key = hashlib.md5(wp.tobytes()).hexdigest()
    if _CACHE.get("key") != key:
        _CACHE["nc"] = build_program(wp)
        _CACHE["key"] = key
    return _CACHE["nc"]


def run_on_cores(nc, in_maps, trace=False, **kwargs):
    from concourse.bass_utils import run_bass_kernel_spmd
    return run_bass_kernel_spmd(
        nc, in_maps, core_ids=list(range(NCORES)), trace=trace, **kwargs)


def _get_runner(nc):
    """Cache the sharded jitted executable so warm kernel() calls skip
    retrace/relower (run_bass_via_pjrt rebuilds its closure per call,
    costing tens of seconds per invocation)."""
    if "runner" in _CACHE:
        return _CACHE["runner"]
    import jax
    from jax.sharding import Mesh, PartitionSpec
    from jax.experimental.shard_map import shard_map
    import concourse.mybir as mybir
    from concourse import bass2jax

    bass2jax.install_neuronx_cc_hook()
    partition_name = nc.partition_id_tensor.name if nc.partition_id_tensor else None
    in_names, out_names, out_avals, zero_outs = [], [], [], []
    for alloc in nc.m.functions[0].allocations:
        if not isinstance(alloc, mybir.MemoryLocationSet):
            continue
        name = alloc.memorylocations[0].name
        if alloc.kind == "ExternalInput":
            if name != partition_name:
                in_names.append(name)
        elif alloc.kind == "ExternalOutput":
            shape = tuple(alloc.tensor_shape)
            dtype = mybir.dt.np(alloc.dtype)
            out_names.append(name)
            out_avals.append(jax.core.ShapedArray(shape, dtype))
            zero_outs.append(np.zeros(shape, dtype))
    all_in_names = list(in_names) + list(out_names)
    if partition_name is not None:
        all_in_names.append(partition_name)

    def _body(*args):
        operands = list(args)
        if partition_name is not None:
            operands.append(bass2jax.partition_id_tensor())
        return tuple(bass2jax._bass_exec_p.bind(
            *operands,
            out_avals=tuple(out_avals),
            in_names=tuple(all_in_names),
            out_names=tuple(out_names),
            lowering_input_output_aliases=(),
            sim_require_finite=True,
            sim_require_nnan=True,
            nc=nc,
        ))

    devices = jax.devices()[:NCORES]
    mesh = Mesh(np.asarray(devices), ("core",))
    nspecs = len(in_names) + len(out_names)
    sharded = jax.jit(
        shard_map(_body, mesh=mesh, in_specs=(PartitionSpec("core"),) * nspecs,
                  out_specs=(PartitionSpec("core"),) * len(out_names),
                  check_rep=False),
        keep_unused=True,
    )
    sharding = jax.sharding.NamedSharding(mesh, PartitionSpec("core"))
    _CACHE["runner"] = (sharded, in_names, zero_outs, sharding)
    return _CACHE["runner"]


def kernel(**inputs) -> np.ndarray:
    import jax
    nc = get_nc(inputs)
    in_maps = prep_inputs(inputs)
    sharded, in_names, zero_outs, sharding = _get_runner(nc)
    concat_in = [
        np.concatenate([np.asarray(in_maps[c][nm]) for c in range(NCORES)], axis=0)
        for nm in in_names
    ]
    concat_zeros = [np.zeros((NCORES * z.shape[0], *z.shape[1:]), z.dtype)
                    for z in zero_outs]
    dev_in = [jax.device_put(a, sharding) for a in concat_in]
    dev_zero = [jax.device_put(a, sharding) for a in concat_zeros]
    outs = sharded(*dev_in, *dev_zero)
    full = np.asarray(outs[0])                     # (512, 768)
    return np.ascontiguousarray(full.reshape(-1, 3).astype(np.float32))
